# revision 6
# baseline (speedup 1.0000x reference)
"""MoE grouped linear (gmm) kernel for 8 Trainium2 NeuronCores.

Strategy (expert parallel, mirrors the shard_map-over-gmm_sharded source):
  - Tokens arrive pre-sorted by expert; group_sizes[e] tokens belong to
    expert e. Core e gets weight[e] plus expert e's token slice, padded to
    MAXG rows so all 8 cores run one SPMD program. The "all-to-all" routing
    is host-side slicing, since kernel() sees the full inputs.
  - Per core we compute y_e^T = W_e^T @ X_e^T (out^T orientation): the
    weight tiles are the PE's stationary operand in natural [K, O] layout
    and X^T (prepared host-side) streams as the moving operand.
  - fp32 inputs are DMA'd untouched into resident SBUF tiles; the PE reads
    the high half of each fp32 word as bf16 through a bitcast + stride-2
    access pattern (truncation toward zero). The mean truncation shrink is
    measured host-side and compensated via the ScalarE evacuation scale;
    the per-partition bias is fused into the same instruction. PSUM
    accumulates in fp32.
Host then unpads/concatenates per-expert outputs back to [T, Out] fp32.
"""

import numpy as np

import concourse.bass as bass
from concourse import bacc
import concourse.mybir as mybir
import concourse.tile as tile
from concourse.bass_utils import run_bass_kernel_spmd

N_CORES = 8
P = 128

_BUILD_CACHE: dict = {}


def _t_chunks(maxg: int) -> list[tuple[int, int]]:
    """Split the token free-dim into PSUM-bank-sized (<=512) chunks."""
    n = (maxg + 511) // 512
    base = ((maxg // n + P - 1) // P) * P
    chunks = []
    off = 0
    while off < maxg:
        sz = min(base, maxg - off)
        chunks.append((off, sz))
        off += sz
    return chunks


def _build_program(maxg: int, n_in: int, n_out: int):
    kb = n_in // P   # contraction blocks
    ob = n_out // P  # output-row blocks
    f32 = mybir.dt.float32
    bf16 = mybir.dt.bfloat16

    nc = bacc.Bacc(
        "TRN2", target_bir_lowering=False, debug=False, num_devices=N_CORES
    )
    xt = nc.dram_tensor("xt", [n_in, maxg], f32, kind="ExternalInput")
    w = nc.dram_tensor("w", [n_in, n_out], f32, kind="ExternalInput")
    bias = nc.dram_tensor("bias", [P, ob], f32, kind="ExternalInput")
    sc = nc.dram_tensor("sc", [P, 1], f32, kind="ExternalInput")
    yt = nc.dram_tensor("yt", [n_out, maxg], f32, kind="ExternalOutput")

    chunks = _t_chunks(maxg)

    with tile.TileContext(nc) as tc:
        with (
            tc.tile_pool(name="const", bufs=1) as constp,
            tc.tile_pool(name="xtsb", bufs=1) as xtp,
            tc.tile_pool(name="wsb", bufs=1) as wp,
            tc.tile_pool(name="outsb", bufs=4) as outp,
            tc.tile_pool(name="psum", bufs=4, space="PSUM") as psump,
        ):
            bias_sb = constp.tile([P, ob], f32)
            nc.sync.dma_start(bias_sb[:], bias[:])
            sc_sb = constp.tile([P, 1], f32)
            nc.sync.dma_start(sc_sb[:], sc[:])

            # All of X^T resident in fp32; bf16 view = high half of each word.
            xt_f32 = xtp.tile([P, kb, maxg], f32)
            for k in range(kb):
                nc.sync.dma_start(xt_f32[:, k, :], xt[k * P : (k + 1) * P, :])
            xv = (
                xt_f32[:]
                .bitcast(bf16)
                .rearrange("p k (t two) -> p k t two", two=2)
            )

            # All of W resident in fp32 (k-major natural slabs), bf16 view.
            w_f32 = wp.tile([P, kb, n_out], f32)
            for k in range(kb):
                nc.sync.dma_start(w_f32[:, k, :], w[k * P : (k + 1) * P, :])
            wv = (
                w_f32[:]
                .bitcast(bf16)
                .rearrange("p k (o two) -> p k o two", two=2)
            )

            for o in range(ob):
                for t0, tsz in chunks:
                    ps = psump.tile([P, tsz], f32, tag="ps")
                    for k in range(kb):
                        nc.tensor.matmul(
                            ps[:],
                            wv[:, k, o * P : (o + 1) * P, 1],
                            xv[:, k, t0 : t0 + tsz, 1],
                            start=(k == 0),
                            stop=(k == kb - 1),
                        )
                    ot = outp.tile([P, tsz], f32, tag="ot")
                    nc.scalar.activation(
                        ot[:],
                        ps[:],
                        mybir.ActivationFunctionType.Identity,
                        bias=bias_sb[:, o : o + 1],
                        scale=sc_sb[:, 0:1],
                    )
                    nc.sync.dma_start(
                        yt[o * P : (o + 1) * P, t0 : t0 + tsz], ot[:]
                    )
    nc.finalize()
    return nc


def _trunc_ratio(a: np.ndarray) -> float:
    """mean(|trunc_bf16(a)|) / mean(|a|) — the systematic shrink from
    reading only the high 16 bits of each fp32."""
    t = (a.view(np.uint32) & np.uint32(0xFFFF0000)).view(np.float32)
    denom = float(np.abs(a).sum())
    if denom == 0.0:
        return 1.0
    return float(np.abs(t).sum()) / denom


def _prepare(inputs, weight, bias, group_sizes):
    """Build (or reuse) the program and the per-core input maps."""
    inputs = np.ascontiguousarray(np.asarray(inputs, dtype=np.float32))
    weight = np.ascontiguousarray(np.asarray(weight, dtype=np.float32))
    bias = np.ascontiguousarray(np.asarray(bias, dtype=np.float32))
    g = np.asarray(group_sizes).astype(np.int64)

    t_tokens, n_in = inputs.shape
    n_exp, _, n_out = weight.shape
    assert n_exp == N_CORES, f"expected {N_CORES} experts, got {n_exp}"
    offs = np.concatenate([[0], np.cumsum(g)])
    assert offs[-1] == t_tokens, "group_sizes must sum to token count"

    maxg = max(P, int(-(-int(g.max()) // P)) * P)

    key = (maxg, n_in, n_out)
    if key not in _BUILD_CACHE:
        _BUILD_CACHE[key] = _build_program(maxg, n_in, n_out)
    nc = _BUILD_CACHE[key]

    ob = n_out // P
    bias_host = np.ascontiguousarray(bias.reshape(ob, P).T)  # [P, ob]

    # Compensate the mean truncation shrink of both operands.
    scale = 1.0 / (_trunc_ratio(inputs) * _trunc_ratio(weight))
    sc_host = np.full((P, 1), scale, np.float32)

    in_maps = []
    for e in range(n_exp):
        xe = inputs[offs[e] : offs[e + 1]]  # [g_e, n_in]
        xt_e = np.zeros((n_in, maxg), np.float32)
        xt_e[:, : g[e]] = xe.T
        in_maps.append(
            {"xt": xt_e, "w": weight[e], "bias": bias_host, "sc": sc_host}
        )
    return nc, in_maps, g, offs, (t_tokens, n_out)


def kernel(inputs, weight, bias, group_sizes):
    nc, in_maps, g, offs, (t_tokens, n_out) = _prepare(
        inputs, weight, bias, group_sizes
    )
    res = run_bass_kernel_spmd(nc, in_maps, core_ids=list(range(N_CORES)))

    out = np.empty((t_tokens, n_out), np.float32)
    for e in range(N_CORES):
        if g[e] == 0:
            continue
        yt_e = res.results[e]["yt"]  # [n_out, maxg]
        out[offs[e] : offs[e + 1]] = yt_e[:, : g[e]].T
    return out


# revision 10
# speedup vs baseline: 1.2828x; 1.2828x over previous
"""MoE grouped linear (gmm) kernel for 8 Trainium2 NeuronCores.

Strategy (expert parallel, mirrors the shard_map-over-gmm_sharded source):
  - Tokens arrive pre-sorted by expert; group_sizes[e] tokens belong to
    expert e. Core e gets weight[e] plus expert e's token slice, padded to
    MAXG rows so all 8 cores run one SPMD program. The "all-to-all" routing
    is host-side slicing, since kernel() sees the full inputs.
  - Per core we compute y_e^T = W_e^T @ X_e^T (out^T orientation): the
    weight tiles are the PE's stationary operand in natural [K, O] layout
    and X^T (prepared host-side) streams as the moving operand.
  - fp32 inputs are DMA'd untouched into resident SBUF tiles; the PE reads
    the high half of each fp32 word as bf16 through a bitcast + stride-2
    access pattern (truncation toward zero). The mean truncation shrink is
    measured host-side and compensated via the ScalarE evacuation scale;
    the per-partition bias is fused into the same instruction. PSUM
    accumulates in fp32.
Host then unpads/concatenates per-expert outputs back to [T, Out] fp32.
"""

import numpy as np

import concourse.bass as bass
from concourse import bacc
import concourse.mybir as mybir
import concourse.tile as tile
from concourse.bass_utils import run_bass_kernel_spmd

N_CORES = 8
P = 128

_BUILD_CACHE: dict = {}


def _t_chunks(maxg: int) -> list[tuple[int, int]]:
    """Split the token free-dim into PSUM-bank-sized (<=512) chunks."""
    n = (maxg + 511) // 512
    base = ((maxg // n + P - 1) // P) * P
    chunks = []
    off = 0
    while off < maxg:
        sz = min(base, maxg - off)
        chunks.append((off, sz))
        off += sz
    return chunks


def _build_program(maxg: int, n_in: int, n_out: int):
    kb = n_in // P   # contraction blocks
    ob = n_out // P  # output-row blocks
    f32 = mybir.dt.float32
    bf16 = mybir.dt.bfloat16

    nc = bacc.Bacc(
        "TRN2", target_bir_lowering=False, debug=False, num_devices=N_CORES
    )
    xt = nc.dram_tensor("xt", [n_in, maxg], f32, kind="ExternalInput")
    w = nc.dram_tensor("w", [ob, n_in, P], f32, kind="ExternalInput")
    bias = nc.dram_tensor("bias", [P, ob], f32, kind="ExternalInput")
    sc = nc.dram_tensor("sc", [P, 1], f32, kind="ExternalInput")
    yt = nc.dram_tensor("yt", [n_out, maxg], f32, kind="ExternalOutput")

    chunks = _t_chunks(maxg)

    with tile.TileContext(nc) as tc:
        with (
            tc.tile_pool(name="const", bufs=1) as constp,
            tc.tile_pool(name="xtsb", bufs=1) as xtp,
            tc.tile_pool(name="wsb", bufs=4) as wp,
            tc.tile_pool(name="outsb", bufs=4) as outp,
            tc.tile_pool(name="psum", bufs=4, space="PSUM") as psump,
        ):
            bias_sb = constp.tile([P, ob], f32)
            nc.sync.dma_start(bias_sb[:], bias[:])
            sc_sb = constp.tile([P, 1], f32)
            nc.sync.dma_start(sc_sb[:], sc[:])

            # All of X^T resident in fp32; bf16 view = high half of each word.
            xt_f32 = xtp.tile([P, kb, maxg], f32)
            for k in range(kb):
                nc.sync.dma_start(xt_f32[:, k, :], xt[k * P : (k + 1) * P, :])
            xv = (
                xt_f32[:]
                .bitcast(bf16)
                .rearrange("p k (t two) -> p k t two", two=2)
            )

            # Stream W one o-slab (all k, 128 output cols) at a time; the
            # slab's bf16 view is the PE stationary operand. Loop order
            # o -> k -> t so one LDWEIGHTS serves every t-chunk.
            for o in range(ob):
                w_o = wp.tile([P, kb, P], f32, tag="wo")
                nc.sync.dma_start(
                    w_o[:], w[o].rearrange("(k p) o -> p k o", p=P)
                )
                wov = (
                    w_o[:]
                    .bitcast(bf16)
                    .rearrange("p k (o two) -> p k o two", two=2)
                )
                pss = [
                    psump.tile(
                        [P, tsz], f32, tag=f"ps{ti}", name=f"ps{o}_{ti}"
                    )
                    for ti, (t0, tsz) in enumerate(chunks)
                ]
                for k in range(kb):
                    for ti, (t0, tsz) in enumerate(chunks):
                        nc.tensor.matmul(
                            pss[ti][:],
                            wov[:, k, :, 1],
                            xv[:, k, t0 : t0 + tsz, 1],
                            start=(k == 0),
                            stop=(k == kb - 1),
                        )
                for ti, (t0, tsz) in enumerate(chunks):
                    ot = outp.tile([P, tsz], f32, tag="ot")
                    nc.scalar.activation(
                        ot[:],
                        pss[ti][:],
                        mybir.ActivationFunctionType.Identity,
                        bias=bias_sb[:, o : o + 1],
                        scale=sc_sb[:, 0:1],
                    )
                    nc.sync.dma_start(
                        yt[o * P : (o + 1) * P, t0 : t0 + tsz], ot[:]
                    )
    nc.finalize()
    return nc


def _trunc_ratio(a: np.ndarray) -> float:
    """mean(|trunc_bf16(a)|) / mean(|a|) — the systematic shrink from
    reading only the high 16 bits of each fp32."""
    t = (a.view(np.uint32) & np.uint32(0xFFFF0000)).view(np.float32)
    denom = float(np.abs(a).sum())
    if denom == 0.0:
        return 1.0
    return float(np.abs(t).sum()) / denom


def _prepare(inputs, weight, bias, group_sizes):
    """Build (or reuse) the program and the per-core input maps."""
    inputs = np.ascontiguousarray(np.asarray(inputs, dtype=np.float32))
    weight = np.ascontiguousarray(np.asarray(weight, dtype=np.float32))
    bias = np.ascontiguousarray(np.asarray(bias, dtype=np.float32))
    g = np.asarray(group_sizes).astype(np.int64)

    t_tokens, n_in = inputs.shape
    n_exp, _, n_out = weight.shape
    assert n_exp == N_CORES, f"expected {N_CORES} experts, got {n_exp}"
    offs = np.concatenate([[0], np.cumsum(g)])
    assert offs[-1] == t_tokens, "group_sizes must sum to token count"

    maxg = max(P, int(-(-int(g.max()) // P)) * P)

    key = (maxg, n_in, n_out)
    if key not in _BUILD_CACHE:
        _BUILD_CACHE[key] = _build_program(maxg, n_in, n_out)
    nc = _BUILD_CACHE[key]

    ob = n_out // P
    bias_host = np.ascontiguousarray(bias.reshape(ob, P).T)  # [P, ob]

    # Compensate the mean truncation shrink of both operands.
    scale = 1.0 / (_trunc_ratio(inputs) * _trunc_ratio(weight))
    sc_host = np.full((P, 1), scale, np.float32)

    in_maps = []
    for e in range(n_exp):
        xe = inputs[offs[e] : offs[e + 1]]  # [g_e, n_in]
        xt_e = np.zeros((n_in, maxg), np.float32)
        xt_e[:, : g[e]] = xe.T
        w_e = np.ascontiguousarray(
            weight[e].reshape(n_in, ob, P).transpose(1, 0, 2)
        )  # [ob, n_in, P]
        in_maps.append(
            {"xt": xt_e, "w": w_e, "bias": bias_host, "sc": sc_host}
        )
    return nc, in_maps, g, offs, (t_tokens, n_out)


def kernel(inputs, weight, bias, group_sizes):
    nc, in_maps, g, offs, (t_tokens, n_out) = _prepare(
        inputs, weight, bias, group_sizes
    )
    res = run_bass_kernel_spmd(nc, in_maps, core_ids=list(range(N_CORES)))

    out = np.empty((t_tokens, n_out), np.float32)
    for e in range(N_CORES):
        if g[e] == 0:
            continue
        yt_e = res.results[e]["yt"]  # [n_out, maxg]
        out[offs[e] : offs[e + 1]] = yt_e[:, : g[e]].T
    return out
